# revision 17
# baseline (speedup 1.0000x reference)
"""Trainium2 Bass kernel for nn_MultiHeadAttention (B=2, L=2048, D=768, H=12).

Sharding: data-parallel over batch (cores 0-3 -> batch 0, cores 4-7 ->
batch 1), tensor-parallel over heads within each batch group (3 heads/core).

Per core:
  x[b] --PE-transpose--> x^T
  x^T  --matmul--> Q^T, K^T (d-major, 64 x L) and V (row-major, L x 64+ones)
  causal flash-style attention with scores kept TRANSPOSED (keys on the
    partition axis, queries on the free axis): softmax-sum is fused into the
    PV matmul via an appended ones column of V; no max-subtraction is needed
    because |scores| stays O(6); the causal mask is applied as a multiply by
    exp(mask) templates on diagonal-straddling key blocks only, fully-masked
    blocks are skipped, fully-unmasked blocks get no mask work.
  AllToAll within the batch group exchanges y^T column slices so every core
    ends up with ALL heads for its own 512 query rows (rank-ordered = head
    -ordered), then a row-sliced output projection produces its slice of the
    final output.

Matmuls run as float32r (TF32-like, full PE rate at N>=256); the tiny
bias/reciprocal broadcast matmuls run as exact float32.
"""
import os
import sys

sys.path.insert(0, "/opt/trn_rl_repo")

VARIANT = set(os.environ.get("KERNEL_VARIANT", "full").split(","))

import numpy as np

import concourse.bacc as bacc
import concourse.mybir as mybir
import concourse.tile as tile
from concourse.bass_utils import run_bass_kernel_spmd
from concourse.masks import make_identity
from contextlib import ExitStack

F32 = mybir.dt.float32
F32R = mybir.dt.float32r
AF = mybir.ActivationFunctionType

B, L, D, H, DK = 2, 2048, 768, 12, 64
N_CORES = 8
HPC = 3          # heads per core
NSL, SL = 4, 512     # query slices per sequence
NKB, KB = 16, 128    # key blocks per sequence
NDC = 6          # D / 128 contraction chunks

_CACHE: dict = {}


def r(ap):
    """View an AP as float32r for full-rate PE matmul."""
    return ap.bitcast(F32R)


def build_kernel(n_copies=1):
    nc = bacc.Bacc("TRN2", target_bir_lowering=False, debug=False,
                   num_devices=N_CORES)

    # ---- external I/O (per-core tensors, prepared host-side) ----
    xb = nc.dram_tensor("xb", [L, D], F32, kind="ExternalInput")
    wqk = nc.dram_tensor("wqk", [NDC, 128, 384], F32R, kind="ExternalInput")
    wv = nc.dram_tensor("wv", [NDC, 128, 256], F32R, kind="ExternalInput")
    bqk = nc.dram_tensor("bqk", [128, 4], F32, kind="ExternalInput")
    bv = nc.dram_tensor("bv", [1, 256], F32, kind="ExternalInput")
    binT = nc.dram_tensor("binT", [4, 128, 2 * SL], F32R, kind="ExternalInput")
    wo = nc.dram_tensor("wo", [D, D], F32R, kind="ExternalInput")
    bo2 = nc.dram_tensor("bo2", [1, D], F32, kind="ExternalInput")
    lcoff = nc.dram_tensor("lcoff", [1, 8], mybir.dt.uint32, kind="ExternalInput")
    out_ext = nc.dram_tensor("out", [1, D] if "tinyout" in VARIANT else [SL, D],
                         F32, kind="ExternalOutput")

    with tile.TileContext(nc) as tc, ExitStack() as top:
        const = top.enter_context(tc.tile_pool(name="const", bufs=1))
        identity = const.tile([128, 128], F32)
        make_identity(nc, identity[:])
        ones = const.tile([1, 128], F32)
        nc.vector.memset(ones[:], 1.0)

        bqk_s = const.tile([128, 4], F32)
        nc.sync.dma_start(bqk_s[:], bqk[:])
        bv_s = const.tile([1, 256], F32)
        nc.sync.dma_start(bv_s[:], bv[:])
        bo_s = const.tile([1, D], F32)
        nc.sync.dma_start(bo_s[:], bo2[:])
        binT_s = const.tile([128, 4, 2 * SL], F32R)
        for o in range(4):
            nc.sync.dma_start(binT_s[:, o, :], binT[o])

        for copy_k in range(n_copies):
            _emit_body(nc, tc, xb, wqk, wv, wo, lcoff, out_ext,
                       identity, ones, bqk_s, bv_s, bo_s, binT_s, copy_k)

    nc.finalize()
    return nc


def _emit_body(nc, tc, xb, wqk, wv, wo, lcoff, out_ext,
               identity, ones, bqk_s, bv_s, bo_s, binT_s, copy_k):
    with ExitStack() as top:
        # persistent per-head tensors
        qkv_pool = top.enter_context(tc.tile_pool(name=f"qkv{copy_k}", bufs=1))
        QQ = qkv_pool.tile([128, L], F32R, name="QQ")   # heads 0|1, d-major
        KK = qkv_pool.tile([128, L], F32R, name="KK")
        Q2 = qkv_pool.tile([64, L], F32R, name="Q2")    # head 2
        K2 = qkv_pool.tile([64, L], F32R, name="K2")
        V_s = [qkv_pool.tile([128, NKB, 65], F32R, name=f"V{j}") for j in range(HPC)]
        for j in range(HPC):
            nc.vector.memset(V_s[j][:, :, 64:65].bitcast(F32), 1.0)
        yT = [qkv_pool.tile([64, L], F32R, name=f"yT{j}") for j in range(HPC)]

        dram = top.enter_context(tc.tile_pool(name=f"dram{copy_k}", bufs=1, space="DRAM"))
        # Chunked AllGather: one collective per query-slice s, overlapped with
        # attention of later slices.  ag_out is chunk-major [4*768, 512].
        ag_in = [dram.tile([HPC * DK, SL], F32R, name=f"agin{s}") for s in range(NSL)]
        ag_out = dram.tile([NSL * 4 * HPC * DK, SL], F32R)

        # ---------- phases: transpose x, project QKV ----------
        with ExitStack() as s1:
            xp = s1.enter_context(tc.tile_pool(name=f"xp{copy_k}", bufs=1))
            xb_t = [xp.tile([128, D], F32, name=f"xb{i}") for i in range(NKB)]
            for i in range(NKB):
                nc.sync.dma_start(xb_t[i][:], xb[128 * i:128 * (i + 1), :])
            xT = [xp.tile([128, L], F32R, name=f"xT{d}") for d in range(NDC)]
            wqk_t = [xp.tile([128, 384], F32R, name=f"wqk{d}") for d in range(NDC)]
            wv_t = [xp.tile([128, 256], F32R, name=f"wv{d}") for d in range(NDC)]
            for d in range(NDC):
                nc.sync.dma_start(wqk_t[d][:], wqk[d])
                nc.sync.dma_start(wv_t[d][:], wv[d])

            pp_t = s1.enter_context(tc.tile_pool(name=f"ppt{copy_k}", bufs=2, space="PSUM"))
            pp_q = s1.enter_context(tc.tile_pool(name=f"ppq{copy_k}", bufs=3, space="PSUM"))
            pp_v = s1.enter_context(tc.tile_pool(name=f"ppv{copy_k}", bufs=2, space="PSUM"))

            # x^T via PE transpose
            for i in range(NKB):
                for d in range(NDC):
                    pt = pp_t.tile([128, 128], F32, tag="pt")
                    nc.tensor.transpose(pt[:], xb_t[i][:, 128 * d:128 * (d + 1)],
                                        identity[:])
                    nc.vector.tensor_copy(xT[d][:, 128 * i:128 * (i + 1)], pt[:])

            # Q/K projections (d-major).  Heads 0,1 paired on partition halves;
            # their attention matmuls later hit different PE row groups and run
            # concurrently.  (in*scale + bias) eviction folds the 1/8 Q scale.
            proj_targets = []
            for s in range(NSL):
                sl = slice(SL * s, SL * (s + 1))
                proj_targets += [
                    (sl, slice(0, 128), [
                        (QQ[0:64, sl], slice(0, 64), bqk_s[0:64, 0:1], 0.125),
                        (QQ[64:128, sl], slice(64, 128), bqk_s[64:128, 0:1], 0.125)]),
                    (sl, slice(128, 256), [
                        (KK[0:64, sl], slice(0, 64), bqk_s[0:64, 1:2], 1.0),
                        (KK[64:128, sl], slice(64, 128), bqk_s[64:128, 1:2], 1.0)]),
                    (sl, slice(256, 320), [
                        (Q2[:, sl], slice(0, 64), bqk_s[0:64, 2:3], 0.125)]),
                    (sl, slice(320, 384), [
                        (K2[:, sl], slice(0, 64), bqk_s[0:64, 3:4], 1.0)]),
                ]
            for sl, wcols, evicts in proj_targets:
                m_par = wcols.stop - wcols.start
                pq = pp_q.tile([128, SL], F32, tag="pq")
                for d in range(NDC):
                    nc.tensor.matmul(pq[0:m_par, :], r(wqk_t[d][:, wcols]),
                                     r(xT[d][:, sl]),
                                     start=(d == 0), stop=(d == NDC - 1))
                for dst, prows, bias_ap, scale in evicts:
                    nc.scalar.activation(dst, pq[prows, :], AF.Identity,
                                         bias=bias_ap, scale=scale)

            # V projection (row-major), bias via ones-row matmul
            for i in range(NKB):
                pv = pp_v.tile([128, 256], F32, tag="pv")
                for d in range(NDC):
                    nc.tensor.matmul(pv[:], r(xT[d][:, 128 * i:128 * (i + 1)]),
                                     r(wv_t[d][:]), start=(d == 0), stop=False)
                nc.tensor.matmul(pv[:], ones[0:1, 0:128], bv_s[:],
                                 start=False, stop=True)
                for j in range(HPC):
                    nc.vector.tensor_copy(V_s[j][:, i, 0:64],
                                          pv[:, 64 * j:64 * (j + 1)])

        # ---------- attention ----------
        def qk_aps(j):
            if j == 0:
                return QQ[0:64, :], KK[0:64, :]
            if j == 1:
                return QQ[64:128, :], KK[64:128, :]
            return Q2[:], K2[:]

        with ExitStack() as s2:
            if "noattn" in VARIANT:
                for j in range(HPC):
                    nc.vector.memset(yT[j][:].bitcast(F32), 0.5)
                raise_skip = True
            pp_s = s2.enter_context(tc.tile_pool(name=f"pps{copy_k}", bufs=2, space="PSUM"))
            pp_y = s2.enter_context(tc.tile_pool(name=f"ppy{copy_k}", bufs=3, space="PSUM"))
            pp_b = s2.enter_context(tc.tile_pool(name=f"ppb{copy_k}", bufs=1, space="PSUM"))
            expp = s2.enter_context(tc.tile_pool(name=f"expp{copy_k}", bufs=3))
            smallp = s2.enter_context(tc.tile_pool(name=f"smallp{copy_k}", bufs=4))

            def normalize(py, j, qsl):
                # yT = py[0:64] * (1 / py[64])
                rec = smallp.tile([1, SL], F32, tag="rec", name="rec")
                nc.vector.reciprocal(rec[:], py[64:65, :])
                pb = pp_b.tile([64, SL], F32, tag="pb", name="pb")
                nc.tensor.matmul(pb[:], ones[0:1, 0:64], rec[:],
                                 start=True, stop=True)
                yt_tmp = smallp.tile([64, SL], F32, tag="ytt", name="ytt")
                nc.scalar.activation(yt_tmp[:], py[0:64, :], AF.Copy)
                nc.vector.tensor_mul(yT[j][:, qsl], yt_tmp[:], pb[:])

            for s in range(0 if "noattn" in VARIANT else NSL):
                qsl = slice(SL * s, SL * (s + 1))
                nkb = 4 * (s + 1)
                # heads 0,1 interleaved at key-block level: their score
                # matmuls use disjoint PE row groups (partitions 0-63 vs
                # 64-127) and run concurrently on the array.
                py0 = pp_y.tile([65, SL], F32, tag="py", name="py0")
                py1 = pp_y.tile([65, SL], F32, tag="py", name="py1")
                for kb in range(nkb):
                    kbsl = slice(128 * kb, 128 * (kb + 1))
                    ps = pp_s.tile([128, 2 * SL], F32, tag="ps", name="ps")
                    nc.tensor.matmul(ps[:, 0:SL], r(KK[0:64, kbsl]),
                                     r(QQ[0:64, qsl]), start=True, stop=True)
                    nc.tensor.matmul(ps[:, SL:2 * SL], r(KK[64:128, kbsl]),
                                     r(QQ[64:128, qsl]), start=True, stop=True)
                    ex = expp.tile([128, 2 * SL], F32R, tag="ex", name="ex")
                    nc.scalar.activation(ex[:], ps[:], AF.Exp)
                    o = kb - 4 * s
                    if o >= 0:  # diagonal-straddling block: apply mask
                        nc.vector.tensor_mul(ex[:], ex[:], binT_s[:, o, :])
                    nc.tensor.matmul(py0[:], r(V_s[0][:, kb, :]), r(ex[:, 0:SL]),
                                     start=(kb == 0), stop=(kb == nkb - 1))
                    nc.tensor.matmul(py1[:], r(V_s[1][:, kb, :]), r(ex[:, SL:2 * SL]),
                                     start=(kb == 0), stop=(kb == nkb - 1))
                normalize(py0, 0, qsl)
                normalize(py1, 1, qsl)
                # head 2: pairs of key blocks
                py2 = pp_y.tile([65, SL], F32, tag="py", name="py2")
                for g0 in range(0, nkb, 2):
                    kbs = [g0, g0 + 1]
                    ps = pp_s.tile([128, 2 * SL], F32, tag="ps", name="ps")
                    for t, kb in enumerate(kbs):
                        nc.tensor.matmul(
                            ps[:, SL * t:SL * (t + 1)],
                            r(K2[:, 128 * kb:128 * (kb + 1)]), r(Q2[:, qsl]),
                            start=True, stop=True)
                    ex = expp.tile([128, 2 * SL], F32R, tag="ex", name="ex")
                    nc.scalar.activation(ex[:], ps[:], AF.Exp)
                    for t, kb in enumerate(kbs):
                        o = kb - 4 * s
                        if o >= 0:
                            nc.vector.tensor_mul(ex[:, SL * t:SL * (t + 1)],
                                                 ex[:, SL * t:SL * (t + 1)],
                                                 binT_s[:, o, 0:SL])
                        nc.tensor.matmul(py2[:], r(V_s[2][:, kb, :]),
                                         r(ex[:, SL * t:SL * (t + 1)]),
                                         start=(kb == 0), stop=(kb == nkb - 1))
                normalize(py2, 2, qsl)
                # AllGather this query-slice chunk across the batch group;
                # runs on TOPSP/SDMA so it overlaps the next slices' compute
                for j in range(HPC):
                    nc.gpsimd.dma_start(ag_in[s][64 * j:64 * (j + 1), :],
                                        yT[j][:, qsl])
                if "noag" in VARIANT:
                    for rep in range(4):
                        nc.gpsimd.dma_start(
                            ag_out[768 * s + 192 * rep:768 * s + 192 * (rep + 1), :],
                            ag_in[s][:])
                else:
                    nc.gpsimd.collective_compute(
                        "AllGather", mybir.AluOpType.bypass,
                        replica_groups=[[0, 1, 2, 3], [4, 5, 6, 7]],
                        ins=[ag_in[s][:].opt()],
                        outs=[ag_out[768 * s:768 * (s + 1), :].opt()])

        # ---------- output projection for this core's 512 query rows ----------
        # The column offset (512 * local-rank) comes from the tiny lcoff input
        # so the single SPMD program serves all cores (dynamic DMA slice).
        with ExitStack() as s3:
            lcp = s3.enter_context(tc.tile_pool(name=f"lcp{copy_k}", bufs=1))
            lc_s = lcp.tile([1, 8], mybir.dt.uint32)
            nc.gpsimd.dma_start(lc_s[:], lcoff[:])
            gp = s3.enter_context(tc.tile_pool(name=f"gp{copy_k}", bufs=1))
            g_t = [gp.tile([128, SL], F32R, name=f"g{d}") for d in range(NDC)]
            wo_t = [gp.tile([128, D], F32R, name=f"wo{d}") for d in range(NDC)]
            from concourse.bass import ds
            for d in range(NDC):
                creg = nc.gpsimd.alloc_register(f"roff_{copy_k}_{d}")
                nc.gpsimd.reg_load(creg, lc_s[0:1, d:d + 1])
                roff = nc.gpsimd.snap(creg, donate=True, min_val=0,
                                      max_val=NSL * 4 * HPC * DK - 128)
                nc.gpsimd.dma_start(g_t[d][:], ag_out[ds(roff, 128), :])
                nc.sync.dma_start(wo_t[d][:], wo[128 * d:128 * (d + 1), :])
            pp_o = s3.enter_context(tc.tile_pool(name=f"ppo{copy_k}", bufs=4, space="PSUM"))
            outp = s3.enter_context(tc.tile_pool(name=f"outp{copy_k}", bufs=2))
            for m in range(4):
                po1 = pp_o.tile([128, SL], F32, tag="po1")
                po2 = pp_o.tile([128, 256], F32, tag="po2")
                for d in range(NDC):
                    lhsT = g_t[d][:, 128 * m:128 * (m + 1)]
                    nc.tensor.matmul(po1[:], r(lhsT), r(wo_t[d][:, 0:512]),
                                     start=(d == 0), stop=False)
                    nc.tensor.matmul(po2[:], r(lhsT), r(wo_t[d][:, 512:768]),
                                     start=(d == 0), stop=False)
                nc.tensor.matmul(po1[:], ones[0:1, 0:128], bo_s[0:1, 0:512],
                                 start=False, stop=True)
                nc.tensor.matmul(po2[:], ones[0:1, 0:128], bo_s[0:1, 512:768],
                                 start=False, stop=True)
                ot = outp.tile([128, D], F32, tag="ot")
                nc.scalar.activation(ot[:, 0:512], po1[:], AF.Copy)
                nc.scalar.activation(ot[:, 512:768], po2[:], AF.Copy)
                if "tinyout" in VARIANT:
                    nc.sync.dma_start(out_ext[0:1, :], ot[0:1, :])
                else:
                    nc.sync.dma_start(out_ext[128 * m:128 * (m + 1), :], ot[:])


def prepare_inputs(x, mask, Wqkv, bqkv, Wo, bo):
    """Build the 8 per-core input maps (host-side slicing/packing)."""
    x = np.asarray(x, np.float32)
    mask = np.asarray(mask, np.float32)
    Wqkv = np.asarray(Wqkv, np.float32)
    bqkv = np.asarray(bqkv, np.float32)
    Wo = np.asarray(Wo, np.float32)
    bo = np.asarray(bo, np.float32)

    # Causal mask templates, taken from the actual mask input (assumes the
    # mask is translation-invariant, which holds for causal masks).  Applied
    # multiplicatively after exp: exp(s + m) == exp(s) * exp(m).
    qs0 = 512
    m2 = mask[0, 0]
    binT = np.empty((4, 128, 2 * SL), np.float32)
    for oi, o in enumerate((0, 128, 256, 384)):
        t = np.exp(m2[qs0:qs0 + SL, qs0 + o:qs0 + o + 128]).T
        binT[oi, :, 0:SL] = t
        binT[oi, :, SL:2 * SL] = t

    wo_c = np.ascontiguousarray(Wo)
    bo_c = bo.reshape(1, D).copy()

    in_maps = []
    for c in range(N_CORES):
        b, lc = c // 4, c % 4
        hs = HPC * lc
        wqk_a = np.empty((NDC, 128, 384), np.float32)
        wv_a = np.zeros((NDC, 128, 256), np.float32)
        for d in range(NDC):
            rows = slice(128 * d, 128 * (d + 1))
            for j in range(HPC):
                h = hs + j
                q_cols = Wqkv[rows, DK * h:DK * (h + 1)]
                k_cols = Wqkv[rows, D + DK * h:D + DK * (h + 1)]
                v_cols = Wqkv[rows, 2 * D + DK * h:2 * D + DK * (h + 1)]
                if j < 2:
                    wqk_a[d, :, 64 * j:64 * (j + 1)] = q_cols
                    wqk_a[d, :, 128 + 64 * j:128 + 64 * (j + 1)] = k_cols
                else:
                    wqk_a[d, :, 256:320] = q_cols
                    wqk_a[d, :, 320:384] = k_cols
                wv_a[d, :, 64 * j:64 * (j + 1)] = v_cols

        bqk_a = np.zeros((128, 4), np.float32)
        bv_a = np.zeros((1, 256), np.float32)
        for j in range(HPC):
            h = hs + j
            bq = bqkv[DK * h:DK * (h + 1)] * 0.125
            bk = bqkv[D + DK * h:D + DK * (h + 1)]
            if j < 2:
                bqk_a[64 * j:64 * (j + 1), 0] = bq
                bqk_a[64 * j:64 * (j + 1), 1] = bk
            else:
                bqk_a[0:64, 2] = bq
                bqk_a[0:64, 3] = bk
            bv_a[0, 64 * j:64 * (j + 1)] = bqkv[2 * D + DK * h:2 * D + DK * (h + 1)]

        in_maps.append({
            "xb": np.ascontiguousarray(x[b]),
            "wqk": wqk_a,
            "wv": wv_a,
            "bqk": bqk_a,
            "bv": bv_a,
            "binT": binT,
            "wo": wo_c,
            "bo2": bo_c,
            "lcoff": np.array([[768 * lc + 128 * d for d in range(6)] + [0, 0]],
                              np.uint32),
        })
    return in_maps


def kernel(**inputs):
    if "nc" not in _CACHE:
        _CACHE["nc"] = build_kernel()
    nc = _CACHE["nc"]
    in_maps = prepare_inputs(inputs["x"], inputs["mask"], inputs["Wqkv"],
                             inputs["bqkv"], inputs["Wo"], inputs["bo"])
    res = run_bass_kernel_spmd(nc, in_maps, core_ids=list(range(N_CORES)))
    out = np.empty((B, L, D), np.float32)
    for c in range(N_CORES):
        b, lc = c // 4, c % 4
        out[b, SL * lc:SL * (lc + 1), :] = res.results[c]["out"]
    return out


# revision 18
# speedup vs baseline: 1.0407x; 1.0407x over previous
"""Trainium2 Bass kernel for nn_MultiHeadAttention (B=2, L=2048, D=768, H=12).

Sharding: data-parallel over batch (cores 0-3 -> batch 0, cores 4-7 ->
batch 1), tensor-parallel over heads within each batch group (3 heads/core).

Per core:
  x[b] --PE-transpose--> x^T
  x^T  --matmul--> Q^T, K^T (d-major, 64 x L) and V (row-major, L x 64+ones)
  causal flash-style attention with scores kept TRANSPOSED (keys on the
    partition axis, queries on the free axis): softmax-sum is fused into the
    PV matmul via an appended ones column of V; no max-subtraction is needed
    because |scores| stays O(6); the causal mask is applied as a multiply by
    exp(mask) templates on diagonal-straddling key blocks only, fully-masked
    blocks are skipped, fully-unmasked blocks get no mask work.
  AllToAll within the batch group exchanges y^T column slices so every core
    ends up with ALL heads for its own 512 query rows (rank-ordered = head
    -ordered), then a row-sliced output projection produces its slice of the
    final output.

Matmuls run as float32r (TF32-like, full PE rate at N>=256); the tiny
bias/reciprocal broadcast matmuls run as exact float32.
"""
import os
import sys

sys.path.insert(0, "/opt/trn_rl_repo")

VARIANT = set(os.environ.get("KERNEL_VARIANT", "full").split(","))

import numpy as np

import concourse.bacc as bacc
import concourse.mybir as mybir
import concourse.tile as tile
from concourse.bass_utils import run_bass_kernel_spmd
from concourse.masks import make_identity
from contextlib import ExitStack

F32 = mybir.dt.float32
F32R = mybir.dt.float32r
AF = mybir.ActivationFunctionType

B, L, D, H, DK = 2, 2048, 768, 12, 64
N_CORES = 8
HPC = 3          # heads per core
NSL, SL = 4, 512     # query slices per sequence
NKB, KB = 16, 128    # key blocks per sequence
NDC = 6          # D / 128 contraction chunks

_CACHE: dict = {}


def r(ap):
    """View an AP as float32r for full-rate PE matmul."""
    return ap.bitcast(F32R)


def build_kernel(n_copies=1):
    nc = bacc.Bacc("TRN2", target_bir_lowering=False, debug=False,
                   num_devices=N_CORES)

    # ---- external I/O (per-core tensors, prepared host-side) ----
    xb = nc.dram_tensor("xb", [L, D], F32, kind="ExternalInput")
    wqk = nc.dram_tensor("wqk", [NDC, 128, 384], F32R, kind="ExternalInput")
    wv = nc.dram_tensor("wv", [NDC, 128, 256], F32R, kind="ExternalInput")
    bqk = nc.dram_tensor("bqk", [128, 4], F32, kind="ExternalInput")
    bv = nc.dram_tensor("bv", [1, 256], F32R, kind="ExternalInput")
    binT = nc.dram_tensor("binT", [4, 128, 2 * SL], F32R, kind="ExternalInput")
    wo = nc.dram_tensor("wo", [D, D], F32R, kind="ExternalInput")
    bo2 = nc.dram_tensor("bo2", [1, D], F32R, kind="ExternalInput")
    lcoff = nc.dram_tensor("lcoff", [1, 8], mybir.dt.uint32, kind="ExternalInput")
    out_ext = nc.dram_tensor("out", [1, D] if "tinyout" in VARIANT else [SL, D],
                         F32, kind="ExternalOutput")

    with tile.TileContext(nc) as tc, ExitStack() as top:
        const = top.enter_context(tc.tile_pool(name="const", bufs=1))
        identity = const.tile([128, 128], F32)
        make_identity(nc, identity[:])
        ones = const.tile([1, 128], F32R)
        nc.vector.memset(ones[:].bitcast(F32), 1.0)

        bqk_s = const.tile([128, 4], F32)
        nc.sync.dma_start(bqk_s[:], bqk[:])
        bv_s = const.tile([1, 256], F32R)
        nc.sync.dma_start(bv_s[:], bv[:])
        bo_s = const.tile([1, D], F32R)
        nc.sync.dma_start(bo_s[:], bo2[:])
        binT_s = const.tile([128, 4, 2 * SL], F32R)
        for o in range(4):
            nc.sync.dma_start(binT_s[:, o, :], binT[o])

        for copy_k in range(n_copies):
            _emit_body(nc, tc, xb, wqk, wv, wo, lcoff, out_ext,
                       identity, ones, bqk_s, bv_s, bo_s, binT_s, copy_k)

    nc.finalize()
    return nc


def _emit_body(nc, tc, xb, wqk, wv, wo, lcoff, out_ext,
               identity, ones, bqk_s, bv_s, bo_s, binT_s, copy_k):
    with ExitStack() as top:
        # persistent per-head tensors
        qkv_pool = top.enter_context(tc.tile_pool(name=f"qkv{copy_k}", bufs=1))
        QQ = qkv_pool.tile([128, L], F32R, name="QQ")   # heads 0|1, d-major
        KK = qkv_pool.tile([128, L], F32R, name="KK")
        Q2 = qkv_pool.tile([64, L], F32R, name="Q2")    # head 2
        K2 = qkv_pool.tile([64, L], F32R, name="K2")
        V_s = [qkv_pool.tile([128, NKB, 65], F32R, name=f"V{j}") for j in range(HPC)]
        for j in range(HPC):
            nc.vector.memset(V_s[j][:, :, 64:65].bitcast(F32), 1.0)
        yT = [qkv_pool.tile([64, L], F32R, name=f"yT{j}") for j in range(HPC)]

        dram = top.enter_context(tc.tile_pool(name=f"dram{copy_k}", bufs=1, space="DRAM"))
        # Chunked AllGather: one collective per query-slice s, overlapped with
        # attention of later slices.  ag_out is chunk-major [4*768, 512].
        ag_in = [dram.tile([HPC * DK, SL], F32R, name=f"agin{s}") for s in range(NSL)]
        ag_out = dram.tile([NSL * 4 * HPC * DK, SL], F32R)

        # ---------- phases: transpose x, project QKV ----------
        with ExitStack() as s1:
            xp = s1.enter_context(tc.tile_pool(name=f"xp{copy_k}", bufs=1))
            xb_t = [xp.tile([128, D], F32, name=f"xb{i}") for i in range(NKB)]
            for i in range(NKB):
                nc.sync.dma_start(xb_t[i][:], xb[128 * i:128 * (i + 1), :])
            xT = [xp.tile([128, L], F32R, name=f"xT{d}") for d in range(NDC)]
            wqk_t = [xp.tile([128, 384], F32R, name=f"wqk{d}") for d in range(NDC)]
            wv_t = [xp.tile([128, 256], F32R, name=f"wv{d}") for d in range(NDC)]
            for d in range(NDC):
                nc.sync.dma_start(wqk_t[d][:], wqk[d])
                nc.sync.dma_start(wv_t[d][:], wv[d])

            pp_t = s1.enter_context(tc.tile_pool(name=f"ppt{copy_k}", bufs=2, space="PSUM"))
            pp_q = s1.enter_context(tc.tile_pool(name=f"ppq{copy_k}", bufs=3, space="PSUM"))
            pp_v = s1.enter_context(tc.tile_pool(name=f"ppv{copy_k}", bufs=2, space="PSUM"))

            # x^T via PE transpose
            for i in range(NKB):
                for d in range(NDC):
                    pt = pp_t.tile([128, 128], F32, tag="pt")
                    nc.tensor.transpose(pt[:], xb_t[i][:, 128 * d:128 * (d + 1)],
                                        identity[:])
                    nc.vector.tensor_copy(xT[d][:, 128 * i:128 * (i + 1)], pt[:])

            # Q/K projections (d-major).  Heads 0,1 paired on partition halves;
            # their attention matmuls later hit different PE row groups and run
            # concurrently.  (in*scale + bias) eviction folds the 1/8 Q scale.
            proj_targets = []
            for s in range(NSL):
                sl = slice(SL * s, SL * (s + 1))
                proj_targets += [
                    (sl, slice(0, 128), [
                        (QQ[0:64, sl], slice(0, 64), bqk_s[0:64, 0:1], 0.125),
                        (QQ[64:128, sl], slice(64, 128), bqk_s[64:128, 0:1], 0.125)]),
                    (sl, slice(128, 256), [
                        (KK[0:64, sl], slice(0, 64), bqk_s[0:64, 1:2], 1.0),
                        (KK[64:128, sl], slice(64, 128), bqk_s[64:128, 1:2], 1.0)]),
                    (sl, slice(256, 320), [
                        (Q2[:, sl], slice(0, 64), bqk_s[0:64, 2:3], 0.125)]),
                    (sl, slice(320, 384), [
                        (K2[:, sl], slice(0, 64), bqk_s[0:64, 3:4], 1.0)]),
                ]
            for sl, wcols, evicts in proj_targets:
                m_par = wcols.stop - wcols.start
                pq = pp_q.tile([128, SL], F32, tag="pq")
                for d in range(NDC):
                    nc.tensor.matmul(pq[0:m_par, :], r(wqk_t[d][:, wcols]),
                                     r(xT[d][:, sl]),
                                     start=(d == 0), stop=(d == NDC - 1))
                for dst, prows, bias_ap, scale in evicts:
                    if scale == 1.0:
                        nc.vector.tensor_scalar_add(dst, pq[prows, :], bias_ap)
                    else:
                        nc.vector.tensor_scalar(dst, pq[prows, :], scale, bias_ap,
                                                mybir.AluOpType.mult,
                                                mybir.AluOpType.add)

            # V projection (row-major), bias via ones-row matmul
            for i in range(NKB):
                pv = pp_v.tile([128, 256], F32, tag="pv")
                for d in range(NDC):
                    nc.tensor.matmul(pv[:], r(xT[d][:, 128 * i:128 * (i + 1)]),
                                     r(wv_t[d][:]), start=(d == 0), stop=False)
                nc.tensor.matmul(pv[:], r(ones[0:1, 0:128]), r(bv_s[:]),
                                 start=False, stop=True)
                for j in range(HPC):
                    nc.vector.tensor_copy(V_s[j][:, i, 0:64],
                                          pv[:, 64 * j:64 * (j + 1)])

        # ---------- attention ----------
        def qk_aps(j):
            if j == 0:
                return QQ[0:64, :], KK[0:64, :]
            if j == 1:
                return QQ[64:128, :], KK[64:128, :]
            return Q2[:], K2[:]

        with ExitStack() as s2:
            if "noattn" in VARIANT:
                for j in range(HPC):
                    nc.vector.memset(yT[j][:].bitcast(F32), 0.5)
                raise_skip = True
            pp_s = s2.enter_context(tc.tile_pool(name=f"pps{copy_k}", bufs=2, space="PSUM"))
            pp_y = s2.enter_context(tc.tile_pool(name=f"ppy{copy_k}", bufs=3, space="PSUM"))
            pp_b = s2.enter_context(tc.tile_pool(name=f"ppb{copy_k}", bufs=1, space="PSUM"))
            expp = s2.enter_context(tc.tile_pool(name=f"expp{copy_k}", bufs=3))
            smallp = s2.enter_context(tc.tile_pool(name=f"smallp{copy_k}", bufs=4))

            def normalize(py, j, qsl):
                # yT = py[0:64] * (1 / py[64])
                rec = smallp.tile([1, SL], F32R, tag="rec", name="rec")
                with nc.allow_low_precision("tf32 softmax-normalize broadcast"):
                    nc.vector.reciprocal(rec[:], py[64:65, :])
                pb = pp_b.tile([64, SL], F32, tag="pb", name="pb")
                nc.tensor.matmul(pb[:], r(ones[0:1, 0:64]), r(rec[:]),
                                 start=True, stop=True)
                yt_tmp = smallp.tile([64, SL], F32, tag="ytt", name="ytt")
                nc.vector.tensor_copy(yt_tmp[:], py[0:64, :])
                nc.vector.tensor_mul(yT[j][:, qsl], yt_tmp[:], pb[:])

            for s in range(0 if "noattn" in VARIANT else NSL):
                qsl = slice(SL * s, SL * (s + 1))
                nkb = 4 * (s + 1)
                # heads 0,1 interleaved at key-block level: their score
                # matmuls use disjoint PE row groups (partitions 0-63 vs
                # 64-127) and run concurrently on the array.
                py0 = pp_y.tile([65, SL], F32, tag="py", name="py0")
                py1 = pp_y.tile([65, SL], F32, tag="py", name="py1")
                for kb in range(nkb):
                    kbsl = slice(128 * kb, 128 * (kb + 1))
                    ps = pp_s.tile([128, 2 * SL], F32, tag="ps", name="ps")
                    nc.tensor.matmul(ps[:, 0:SL], r(KK[0:64, kbsl]),
                                     r(QQ[0:64, qsl]), start=True, stop=True)
                    nc.tensor.matmul(ps[:, SL:2 * SL], r(KK[64:128, kbsl]),
                                     r(QQ[64:128, qsl]), start=True, stop=True)
                    ex = expp.tile([128, 2 * SL], F32R, tag="ex", name="ex")
                    nc.scalar.activation(ex[:], ps[:], AF.Exp)
                    o = kb - 4 * s
                    if o >= 0:  # diagonal-straddling block: apply mask
                        nc.vector.tensor_mul(ex[:], ex[:], binT_s[:, o, :])
                    nc.tensor.matmul(py0[:], r(V_s[0][:, kb, :]), r(ex[:, 0:SL]),
                                     start=(kb == 0), stop=(kb == nkb - 1))
                    nc.tensor.matmul(py1[:], r(V_s[1][:, kb, :]), r(ex[:, SL:2 * SL]),
                                     start=(kb == 0), stop=(kb == nkb - 1))
                normalize(py0, 0, qsl)
                normalize(py1, 1, qsl)
                # head 2: pairs of key blocks
                py2 = pp_y.tile([65, SL], F32, tag="py", name="py2")
                for g0 in range(0, nkb, 2):
                    kbs = [g0, g0 + 1]
                    ps = pp_s.tile([128, 2 * SL], F32, tag="ps", name="ps")
                    for t, kb in enumerate(kbs):
                        nc.tensor.matmul(
                            ps[:, SL * t:SL * (t + 1)],
                            r(K2[:, 128 * kb:128 * (kb + 1)]), r(Q2[:, qsl]),
                            start=True, stop=True)
                    ex = expp.tile([128, 2 * SL], F32R, tag="ex", name="ex")
                    nc.scalar.activation(ex[:], ps[:], AF.Exp)
                    for t, kb in enumerate(kbs):
                        o = kb - 4 * s
                        if o >= 0:
                            nc.vector.tensor_mul(ex[:, SL * t:SL * (t + 1)],
                                                 ex[:, SL * t:SL * (t + 1)],
                                                 binT_s[:, o, 0:SL])
                        nc.tensor.matmul(py2[:], r(V_s[2][:, kb, :]),
                                         r(ex[:, SL * t:SL * (t + 1)]),
                                         start=(kb == 0), stop=(kb == nkb - 1))
                normalize(py2, 2, qsl)
                # AllGather this query-slice chunk across the batch group;
                # runs on TOPSP/SDMA so it overlaps the next slices' compute
                for j in range(HPC):
                    nc.gpsimd.dma_start(ag_in[s][64 * j:64 * (j + 1), :],
                                        yT[j][:, qsl])
                if "noag" in VARIANT:
                    for rep in range(4):
                        nc.gpsimd.dma_start(
                            ag_out[768 * s + 192 * rep:768 * s + 192 * (rep + 1), :],
                            ag_in[s][:])
                else:
                    nc.gpsimd.collective_compute(
                        "AllGather", mybir.AluOpType.bypass,
                        replica_groups=[[0, 1, 2, 3], [4, 5, 6, 7]],
                        ins=[ag_in[s][:].opt()],
                        outs=[ag_out[768 * s:768 * (s + 1), :].opt()])

        # ---------- output projection for this core's 512 query rows ----------
        # The column offset (512 * local-rank) comes from the tiny lcoff input
        # so the single SPMD program serves all cores (dynamic DMA slice).
        with ExitStack() as s3:
            lcp = s3.enter_context(tc.tile_pool(name=f"lcp{copy_k}", bufs=1))
            lc_s = lcp.tile([1, 8], mybir.dt.uint32)
            nc.gpsimd.dma_start(lc_s[:], lcoff[:])
            gp = s3.enter_context(tc.tile_pool(name=f"gp{copy_k}", bufs=1))
            g_t = [gp.tile([128, SL], F32R, name=f"g{d}") for d in range(NDC)]
            wo_t = [gp.tile([128, D], F32R, name=f"wo{d}") for d in range(NDC)]
            from concourse.bass import ds
            for d in range(NDC):
                creg = nc.gpsimd.alloc_register(f"roff_{copy_k}_{d}")
                nc.gpsimd.reg_load(creg, lc_s[0:1, d:d + 1])
                roff = nc.gpsimd.snap(creg, donate=True, min_val=0,
                                      max_val=NSL * 4 * HPC * DK - 128)
                nc.gpsimd.dma_start(g_t[d][:], ag_out[ds(roff, 128), :])
                nc.sync.dma_start(wo_t[d][:], wo[128 * d:128 * (d + 1), :])
            pp_o = s3.enter_context(tc.tile_pool(name=f"ppo{copy_k}", bufs=4, space="PSUM"))
            outp = s3.enter_context(tc.tile_pool(name=f"outp{copy_k}", bufs=2))
            for m in range(4):
                po1 = pp_o.tile([128, SL], F32, tag="po1")
                po2 = pp_o.tile([128, 256], F32, tag="po2")
                for d in range(NDC):
                    lhsT = g_t[d][:, 128 * m:128 * (m + 1)]
                    nc.tensor.matmul(po1[:], r(lhsT), r(wo_t[d][:, 0:512]),
                                     start=(d == 0), stop=False)
                    nc.tensor.matmul(po2[:], r(lhsT), r(wo_t[d][:, 512:768]),
                                     start=(d == 0), stop=False)
                nc.tensor.matmul(po1[:], r(ones[0:1, 0:128]), r(bo_s[0:1, 0:512]),
                                 start=False, stop=True)
                nc.tensor.matmul(po2[:], r(ones[0:1, 0:128]), r(bo_s[0:1, 512:768]),
                                 start=False, stop=True)
                ot = outp.tile([128, D], F32, tag="ot")
                nc.scalar.activation(ot[:, 0:512], po1[:], AF.Copy)
                nc.scalar.activation(ot[:, 512:768], po2[:], AF.Copy)
                if "tinyout" in VARIANT:
                    nc.sync.dma_start(out_ext[0:1, :], ot[0:1, :])
                else:
                    nc.sync.dma_start(out_ext[128 * m:128 * (m + 1), :], ot[:])


def prepare_inputs(x, mask, Wqkv, bqkv, Wo, bo):
    """Build the 8 per-core input maps (host-side slicing/packing)."""
    x = np.asarray(x, np.float32)
    mask = np.asarray(mask, np.float32)
    Wqkv = np.asarray(Wqkv, np.float32)
    bqkv = np.asarray(bqkv, np.float32)
    Wo = np.asarray(Wo, np.float32)
    bo = np.asarray(bo, np.float32)

    # Causal mask templates, taken from the actual mask input (assumes the
    # mask is translation-invariant, which holds for causal masks).  Applied
    # multiplicatively after exp: exp(s + m) == exp(s) * exp(m).
    qs0 = 512
    m2 = mask[0, 0]
    binT = np.empty((4, 128, 2 * SL), np.float32)
    for oi, o in enumerate((0, 128, 256, 384)):
        t = np.exp(m2[qs0:qs0 + SL, qs0 + o:qs0 + o + 128]).T
        binT[oi, :, 0:SL] = t
        binT[oi, :, SL:2 * SL] = t

    wo_c = np.ascontiguousarray(Wo)
    bo_c = bo.reshape(1, D).copy()

    in_maps = []
    for c in range(N_CORES):
        b, lc = c // 4, c % 4
        hs = HPC * lc
        wqk_a = np.empty((NDC, 128, 384), np.float32)
        wv_a = np.zeros((NDC, 128, 256), np.float32)
        for d in range(NDC):
            rows = slice(128 * d, 128 * (d + 1))
            for j in range(HPC):
                h = hs + j
                q_cols = Wqkv[rows, DK * h:DK * (h + 1)]
                k_cols = Wqkv[rows, D + DK * h:D + DK * (h + 1)]
                v_cols = Wqkv[rows, 2 * D + DK * h:2 * D + DK * (h + 1)]
                if j < 2:
                    wqk_a[d, :, 64 * j:64 * (j + 1)] = q_cols
                    wqk_a[d, :, 128 + 64 * j:128 + 64 * (j + 1)] = k_cols
                else:
                    wqk_a[d, :, 256:320] = q_cols
                    wqk_a[d, :, 320:384] = k_cols
                wv_a[d, :, 64 * j:64 * (j + 1)] = v_cols

        bqk_a = np.zeros((128, 4), np.float32)
        bv_a = np.zeros((1, 256), np.float32)
        for j in range(HPC):
            h = hs + j
            bq = bqkv[DK * h:DK * (h + 1)] * 0.125
            bk = bqkv[D + DK * h:D + DK * (h + 1)]
            if j < 2:
                bqk_a[64 * j:64 * (j + 1), 0] = bq
                bqk_a[64 * j:64 * (j + 1), 1] = bk
            else:
                bqk_a[0:64, 2] = bq
                bqk_a[0:64, 3] = bk
            bv_a[0, 64 * j:64 * (j + 1)] = bqkv[2 * D + DK * h:2 * D + DK * (h + 1)]

        in_maps.append({
            "xb": np.ascontiguousarray(x[b]),
            "wqk": wqk_a,
            "wv": wv_a,
            "bqk": bqk_a,
            "bv": bv_a,
            "binT": binT,
            "wo": wo_c,
            "bo2": bo_c,
            "lcoff": np.array([[768 * lc + 128 * d for d in range(6)] + [0, 0]],
                              np.uint32),
        })
    return in_maps


def kernel(**inputs):
    if "nc" not in _CACHE:
        _CACHE["nc"] = build_kernel()
    nc = _CACHE["nc"]
    in_maps = prepare_inputs(inputs["x"], inputs["mask"], inputs["Wqkv"],
                             inputs["bqkv"], inputs["Wo"], inputs["bo"])
    res = run_bass_kernel_spmd(nc, in_maps, core_ids=list(range(N_CORES)))
    out = np.empty((B, L, D), np.float32)
    for c in range(N_CORES):
        b, lc = c // 4, c % 4
        out[b, SL * lc:SL * (lc + 1), :] = res.results[c]["out"]
    return out
